# revision 1
# baseline (speedup 1.0000x reference)
"""Multi-head causal self-attention block for Trainium2, data-parallel over 8 cores.

Reference computation (per batch b of x [B=32, T=1024, C=384]):
    qkv = x @ W_attn;  q,k,v heads (H=6, D=64)
    y   = softmax(causal(q k^T / sqrt(D))) @ v
    out = y @ W_proj + b_proj
Sharding: batch dim 32 -> 4 per core, weights replicated, no collectives.

v3 design (bf16 matmul operands, f32 psum accumulation; rel tol is 2e-2 so
bf16 rounding is safe):
  - x^T via PE transposes (f32), evicted to bf16.
  - q^T,k^T per head-pair [128, T] (W_attn slices stationary); v natural
    [tok, 384] per k-tile with a bf16 ones column per head.
  - scores transposed s^T[k,q] per 512-wide q chunk, exp on ACT (scale=1/8)
    into bf16 pT; causal diagonal fixed by gpsimd affine_select.
  - AV in NATURAL layout: out[q, d+1] accumulated per 128-q chunk with
    lhsT = pT chunk, rhs = v_aug [128, 65]; one sequential psum accumulation
    chain per (q-chunk, head) region (one pending group per 2KB psum bank).
    The softmax denominator lands per-PARTITION, so normalization is a
    strided DVE reciprocal + broadcast-AP multiplies -- no DMA bounce.
  - y natural -> y^T via PE transposes (bf16 into psum), then proj with the
    bias added on the DVE eviction.
  - INTERLEAVED EMISSION: the attention inner loop is ACT(exp)-bound, so
    between score/exp steps we emit "filler" chunks -- proj(b-1) chains and
    prep(b+1) chunks (x-transposes, v, qk) -- keeping the in-order PE queue
    dense enough that the tensor engine stays ramped at its top p-state.
  - v-aug eviction on gpsimd (otherwise idle) to unload ACT/DVE.
"""

import sys

if "/opt/trn_rl_repo" not in sys.path:
    sys.path.insert(0, "/opt/trn_rl_repo")

import numpy as np

B, T, C = 32, 1024, 384
H, D = 6, 64
NCORES = 8
BPC = B // NCORES          # batches per core
NPAIR = H // 2             # head pairs
TT = T // 128              # token tiles per batch (8)
QC = T // 512              # q chunks per batch (2)
VSTRIDE = H * (D + 1)      # 390: per-token-tile v_aug row width

_nc_cache = {}


def _build_nc():
    import concourse.mybir as mybir
    from concourse import bacc
    from concourse.tile import TileContext
    from concourse.masks import make_identity

    f32 = mybir.dt.float32
    bf16 = mybir.dt.bfloat16
    Exp = mybir.ActivationFunctionType.Exp
    Copy = mybir.ActivationFunctionType.Copy
    GE = mybir.AluOpType.is_ge

    nc = bacc.Bacc("TRN2", target_bir_lowering=False, debug=False, num_devices=NCORES)

    x_d = nc.declare_dram_parameter("x", [BPC, T, C], f32, isOutput=False)
    wa_d = nc.declare_dram_parameter("W_attn", [C, 3 * C], f32, isOutput=False)
    wp_d = nc.declare_dram_parameter("W_proj", [C, C], f32, isOutput=False)
    bp_d = nc.declare_dram_parameter("b_proj", [C], f32, isOutput=False)
    out_d = nc.declare_dram_parameter("out", [BPC, T, C], f32, isOutput=True)

    with TileContext(nc) as tc:
        with (
            tc.tile_pool(name="const", bufs=1) as const,
            tc.tile_pool(name="xin", bufs=2) as xin,
            tc.tile_pool(name="xT", bufs=2) as xTp,
            tc.tile_pool(name="qk", bufs=2) as qkp,
            tc.tile_pool(name="vb", bufs=2) as vbp,
            tc.tile_pool(name="pT", bufs=2) as pTp,
            tc.tile_pool(name="yn", bufs=2) as ynp,
            tc.tile_pool(name="yT", bufs=2) as yTp,
            tc.tile_pool(name="rc", bufs=4) as rcp,
            tc.tile_pool(name="osb", bufs=2) as osbp,
            tc.tile_pool(name="psA", bufs=2, space="PSUM") as psA,   # 1 bank each
            tc.tile_pool(name="psS", bufs=2, space="PSUM") as psS,   # 2 banks each
            tc.tile_pool(name="psY", bufs=1, space="PSUM") as psY,   # 2 banks
        ):
            ident = const.tile([128, 128], f32, tag="ident")
            make_identity(nc, ident[:])
            identb = const.tile([128, 128], bf16, tag="identb")
            nc.vector.tensor_copy(identb[:], ident[:])

            def load(b):
                xb = xin.tile([128, TT * C], f32, tag="xin", name=f"xb{b}")
                for t in range(TT):
                    nc.sync.dma_start(
                        out=xb[:, t * C:(t + 1) * C],
                        in_=x_d[b, t * 128:(t + 1) * 128, :],
                    )
                return xb

            xb0_early = load(0)

            wa_sb = []
            wp_sb = []
            for c in range(3):
                wf = xin.tile([128, 4 * C], f32, tag="wf", name=f"wf{c}")
                nc.sync.dma_start(out=wf[:, 0: 3 * C],
                                  in_=wa_d[c * 128:(c + 1) * 128, :])
                nc.sync.dma_start(out=wf[:, 3 * C: 4 * C],
                                  in_=wp_d[c * 128:(c + 1) * 128, :])
                w = const.tile([128, 3 * C], bf16, tag=f"wa{c}")
                nc.vector.tensor_copy(w[:], wf[:, 0: 3 * C])
                wa_sb.append(w)
                p = const.tile([128, C], bf16, tag=f"wp{c}")
                nc.vector.tensor_copy(p[:], wf[:, 3 * C: 4 * C])
                wp_sb.append(p)
            b_bc = const.tile([128, C], f32, tag="bbc")
            nc.sync.dma_start(
                out=b_bc[:], in_=bp_d[:].unsqueeze(0).broadcast_to([128, C])
            )

            def emit_xtr(b, xb, xT, c, half):
                pst = psA.tile([128, 512], f32, tag="psA", name=f"pst{b}")
                for j in range(4):
                    nc.tensor.transpose(
                        pst[:, j * 128:(j + 1) * 128],
                        xb[:, (half * 4 + j) * C + c * 128:
                           (half * 4 + j) * C + c * 128 + 128],
                        ident[:],
                    )
                nc.vector.tensor_copy(
                    xT[:, c * T + half * 512: c * T + half * 512 + 512],
                    pst[:],
                )

            def emit_v(b, xT, vb, t):
                psv = psA.tile([128, 512], f32, tag="psA", name=f"psv{b}")
                for c in range(3):
                    nc.tensor.matmul(
                        psv[:, 0:C],
                        lhsT=xT[:, c * T + t * 128: c * T + t * 128 + 128],
                        rhs=wa_sb[c][:, 2 * C: 3 * C],
                        start=(c == 0),
                        stop=(c == 2),
                    )
                nc.vector.tensor_copy(
                    vb[:, t * VSTRIDE: t * VSTRIDE + VSTRIDE]
                    .rearrange("p (h e) -> p h e", e=D + 1)[:, :, 0:D],
                    psv[:, 0:C].rearrange("p (h d) -> p h d", d=D),
                )

            def emit_qk(b, xT, qk, i, m, half):
                psq = psA.tile([128, 512], f32, tag="psA", name=f"psq{b}")
                for c in range(3):
                    nc.tensor.matmul(
                        psq[:],
                        lhsT=wa_sb[c][:, m * 128:(m + 1) * 128],
                        rhs=xT[:, c * T + half * 512: c * T + half * 512 + 512],
                        start=(c == 0),
                        stop=(c == 2),
                    )
                nc.vector.tensor_copy(
                    qk[:, i * T + half * 512: i * T + half * 512 + 512],
                    psq[:],
                )

            def prep_tiles(b):
                xT = xTp.tile([128, 3 * T], bf16, tag="xT", name=f"xT{b}")
                vb = vbp.tile([128, TT * VSTRIDE], bf16, tag="vb", name=f"vb{b}")
                qks = [qkp.tile([128, 2 * T], bf16, tag=f"qk{pp}",
                                name=f"qk{b}_{pp}") for pp in range(NPAIR)]
                return xT, vb, qks

            def prep_fillers(b, xb):
                """Closure list building xT, v_aug and q^T/k^T for batch b."""
                xT, vb, qks = prep_tiles(b)
                fl = []
                for half in range(2):
                    for c in range(3):
                        fl.append(lambda c=c, half=half:
                                  emit_xtr(b, xb, xT, c, half))

                def vb_init():
                    nc.gpsimd.memset(
                        vb[:].rearrange("p (t h e) -> p t h e", t=TT, e=D + 1)
                        [:, :, :, D:],
                        1.0,
                    )
                fl.append(vb_init)

                vq = []
                for t in range(TT):
                    vq.append(lambda t=t: emit_v(b, xT, vb, t))
                qq = []
                for pp in range(NPAIR):
                    for i, m in enumerate((pp, 3 + pp)):
                        for half in range(2):
                            qq.append(lambda pp=pp, i=i, m=m, half=half:
                                      emit_qk(b, xT, qks[pp], i, m, half))
                # interleave v and qk chunks
                while vq or qq:
                    if vq:
                        fl.append(vq.pop(0))
                    if qq:
                        fl.append(qq.pop(0))
                    if qq:
                        fl.append(qq.pop(0))
                return vb, qks, fl

            def emit_proj(b, yT, t):
                pso = psA.tile([128, 512], f32, tag="psA", name=f"pso{b}")
                for c in range(3):
                    nc.tensor.matmul(
                        pso[:, 0:C],
                        lhsT=yT[:, c * T + t * 128: c * T + t * 128 + 128],
                        rhs=wp_sb[c][:],
                        start=(c == 0),
                        stop=(c == 2),
                    )
                osb = osbp.tile([128, C], f32, tag="osb", name=f"osb{b}")
                nc.vector.tensor_add(osb[:], pso[:, 0:C], b_bc[:])
                nc.sync.dma_start(
                    out=out_d[b, t * 128:(t + 1) * 128, :], in_=osb[:]
                )

            def attn(b, vb, qks, fill):
                """Attention for batch b -> y^T [128(c), 3*T] bf16."""
                yT = yTp.tile([128, 3 * T], bf16, tag="yT", name=f"yT{b}")
                for qc in range(QC):
                    yn = ynp.tile([128, 4 * C], bf16, tag=f"yn{qc}",
                                  name=f"yn{b}_{qc}")
                    for p in range(NPAIR):
                        qk = qks[p]
                        nkt = 4 * (qc + 1)
                        ptw = nkt * 512
                        pT = pTp.tile([128, 2 * ptw], bf16, tag=f"pT{qc}",
                                      name=f"pT{b}_{p}_{qc}")
                        ys = psY.tile([128, 1024], f32, tag="psY",
                                      name=f"ys{b}{p}{qc}")

                        def ycol(j, hh):
                            return (512 if j == 3 else j * 130) + hh * 65

                        def emit_scores_exp(kt):
                            diag = kt >= qc * 4
                            o = (kt - qc * 4) * 128 if diag else 0
                            pss = psS.tile([128, 1024], f32, tag="psS",
                                           name=f"pss{b}{p}")
                            for hh in range(2):
                                nc.tensor.matmul(
                                    pss[:, hh * 512 + o:(hh + 1) * 512],
                                    lhsT=qk[hh * 64:(hh + 1) * 64,
                                            T + kt * 128: T + kt * 128 + 128],
                                    rhs=qk[hh * 64:(hh + 1) * 64,
                                           qc * 512 + o: qc * 512 + 512],
                                    start=True,
                                    stop=True,
                                )
                            nc.scalar.activation(
                                pT[:].rearrange("p (h w) -> p h w", h=2)
                                [:, :, kt * 512 + o: (kt + 1) * 512],
                                pss[:].rearrange("p (h w) -> p h w", h=2)
                                [:, :, o:512],
                                Exp,
                                scale=0.125,
                            )
                            if diag:
                                blk = pT[:].rearrange("p (h w) -> p h w", h=2)[
                                    :, :, kt * 512 + o: kt * 512 + o + 128]
                                nc.gpsimd.affine_select(
                                    out=blk,
                                    in_=blk,
                                    compare_op=GE,
                                    fill=0.0,
                                    base=0,
                                    pattern=[[0, 2], [1, 128]],
                                    channel_multiplier=-1,
                                )

                        def emit_y(j):
                            # one accumulation chain per (j, hh) psum region;
                            # chains strictly sequential within a psum bank
                            qt = qc * 4 + j
                            for hh in range(2):
                                h = 2 * p + hh
                                for k2 in range(qt + 1):
                                    nc.tensor.matmul(
                                        ys[:, ycol(j, hh): ycol(j, hh) + 65],
                                        lhsT=pT[:, hh * ptw + k2 * 512 + j * 128:
                                                hh * ptw + k2 * 512 + j * 128 + 128],
                                        rhs=vb[:, k2 * VSTRIDE + h * (D + 1):
                                               k2 * VSTRIDE + (h + 1) * (D + 1)],
                                        start=(k2 == 0),
                                        stop=(k2 == qt),
                                    )

                        quota = 3 if qc == 0 else 5
                        emit_scores_exp(0)
                        if quota:
                            fill(1)
                            quota -= 1
                        for kt in range(1, nkt):
                            emit_scores_exp(kt)
                            if quota:
                                fill(1)
                                quota -= 1
                            if kt - 1 >= qc * 4:
                                emit_y(kt - 1 - qc * 4)
                        emit_y(3)

                        rc = rcp.tile([128, 8], f32, tag="rc",
                                      name=f"rc{b}{p}{qc}")
                        nc.vector.reciprocal(rc[:, 0:6], ys[:, 64:454:65])
                        nc.vector.reciprocal(rc[:, 6:8], ys[:, 576:706:65])
                        nc.vector.tensor_mul(
                            yn[:, 0: 3 * C]
                            .rearrange("p (j w) -> p j w", j=3)
                            [:, :, 2 * p * 64: 2 * p * 64 + 128]
                            .rearrange("p j (g e) -> p j g e", e=D),
                            ys[:, 0:390]
                            .rearrange("p (j g e) -> p j g e", g=2, e=D + 1)
                            [:, :, :, 0:D],
                            rc[:, 0:6]
                            .rearrange("p (j g) -> p j g", g=2)
                            .unsqueeze(3).broadcast_to([128, 3, 2, D]),
                        )
                        nc.vector.tensor_mul(
                            yn[:, 3 * C + 2 * p * 64: 3 * C + 2 * p * 64 + 128]
                            .rearrange("p (g e) -> p g e", e=D),
                            ys[:, 512:642]
                            .rearrange("p (g e) -> p g e", e=D + 1)[:, :, 0:D],
                            rc[:, 6:8].unsqueeze(2).broadcast_to([128, 2, D]),
                        )
                        fill(1)

                    # transpose y for this qc, with each q-tile's projection
                    # emitted one step behind its transpose eviction
                    for j in range(4):
                        qt = qc * 4 + j
                        psT = psA.tile([128, 1024], bf16, tag="psA",
                                       name=f"psT{b}")
                        for c in range(3):
                            nc.tensor.transpose(
                                psT[:, c * 128:(c + 1) * 128],
                                yn[:, j * C + c * 128: j * C + c * 128 + 128],
                                identb[:],
                            )
                        nc.vector.tensor_copy(
                            yT[:].rearrange("p (c t) -> p c t", c=3)
                            [:, :, qt * 128: qt * 128 + 128],
                            psT[:, 0:C].rearrange("p (c e) -> p c e", e=128),
                        )
                        if j > 0:
                            emit_proj(b, yT, qc * 4 + j - 1)
                        else:
                            fill(1)
                    emit_proj(b, yT, qc * 4 + 3)
                return yT

            # ---- schedule ----
            fillers = []

            def fill(n=1):
                for _ in range(n):
                    if fillers:
                        fillers.pop(0)()

            vb0, qks0, fl0 = prep_fillers(0, xb0_early)
            for f in fl0:
                f()
            xb_next = load(1)

            vb_cur, qks_cur = vb0, qks0
            for b in range(BPC):
                if b + 1 < BPC:
                    vb_nxt, qks_nxt, flp = prep_fillers(b + 1, xb_next)
                    fillers = flp
                else:
                    fillers = []
                if b + 2 < BPC:
                    xb_next = load(b + 2)
                attn(b, vb_cur, qks_cur, fill)
                while fillers:
                    fillers.pop(0)()
                if b + 1 < BPC:
                    vb_cur, qks_cur = vb_nxt, qks_nxt

    nc.finalize()
    return nc


def _run(inputs, trace=False, **kw):
    from concourse.bass_utils import run_bass_kernel_spmd

    if "nc" not in _nc_cache:
        _nc_cache["nc"] = _build_nc()
    nc = _nc_cache["nc"]

    x = np.ascontiguousarray(np.asarray(inputs["x"], dtype=np.float32))
    wa = np.ascontiguousarray(np.asarray(inputs["W_attn"], dtype=np.float32))
    wp = np.ascontiguousarray(np.asarray(inputs["W_proj"], dtype=np.float32))
    bp = np.ascontiguousarray(np.asarray(inputs["b_proj"], dtype=np.float32))

    in_maps = [
        {"x": x[i * BPC:(i + 1) * BPC], "W_attn": wa, "W_proj": wp, "b_proj": bp}
        for i in range(NCORES)
    ]
    res = run_bass_kernel_spmd(nc, in_maps, list(range(NCORES)), trace=trace, **kw)
    out = np.concatenate([res.results[i]["out"] for i in range(NCORES)], axis=0)
    return out, res


def kernel(**inputs) -> np.ndarray:
    out, _ = _run(inputs, trace=False)
    return out



# revision 4
# speedup vs baseline: 1.0308x; 1.0308x over previous
"""Multi-head causal self-attention block for Trainium2, data-parallel over 8 cores.

Reference computation (per batch b of x [B=32, T=1024, C=384]):
    qkv = x @ W_attn;  q,k,v heads (H=6, D=64)
    y   = softmax(causal(q k^T / sqrt(D))) @ v
    out = y @ W_proj + b_proj
Sharding: batch dim 32 -> 4 per core, weights replicated, no collectives.

v4 design (evolves v3; bf16 matmul operands, f32 psum accumulation):
  - x loaded as bf16 via ONE gpsimd cast-DMA per batch (SWDGE casts f32->bf16
    in flight); x^T built by DMA crossbar transposes (dma_start_transpose) --
    the PE does no transposes at all and the DVE does no xT evictions.
  - weights also arrive as bf16 via cast-DMA (no staging/cast instructions).
  - q^T,k^T per head-pair [128, T] (W_attn slices stationary); v natural
    [tok, 384] per k-tile with a bf16 ones column per head (softmax
    denominator rides the AV matmul for free).
  - scores transposed s^T[k,q] per 512-wide q chunk, exp on ACT (scale=1/8)
    into bf16 pT; causal diagonal fixed by gpsimd affine_select.
  - AV in NATURAL layout: out[q, d+1] accumulated per 128-q chunk; softmax
    normalization = strided DVE reciprocal + broadcast-AP multiplies.
  - y natural -> y^T via DMA crossbar transposes (no PE/DVE involvement);
    proj runs as deferred filler work so the crossbar latency hides behind
    the next q-chunk's score phase; bias added on the DVE eviction.
  - VIRTUAL-CLOCK INTERLEAVE: emission tracks estimated cumulative PE and
    ACT busy-ns; filler chunks (next batch's v/qk, deferred proj) are
    inserted whenever the PE stream would fall behind the ACT stream,
    keeping the in-order PE queue dense so the tensor engine stays ramped
    at its top p-state. Fillers carry deadline tags (global qc index) and
    are force-emitted when their consumer phase begins. Per-batch prep is
    split into an "early" part (needed before the batch's qc0 attention)
    and a "late" part (only needed by qc1), widening the filler window.
"""

import sys

if "/opt/trn_rl_repo" not in sys.path:
    sys.path.insert(0, "/opt/trn_rl_repo")

import numpy as np

B, T, C = 32, 1024, 384
H, D = 6, 64
NCORES = 8
BPC = B // NCORES          # batches per core
NPAIR = H // 2             # head pairs
TT = T // 128              # token tiles per batch (8)
QC = T // 512              # q chunks per batch (2)
VSTRIDE = H * (D + 1)      # 390: per-token-tile v_aug row width

VM_MARGIN = 500.0          # ns of PE lead to maintain over ACT
VM_CAP = 1500.0            # max bankable PE lead (queue depth model)

_nc_cache = {}


def _build_nc():
    import concourse.mybir as mybir
    from concourse import bacc
    from concourse.tile import TileContext

    f32 = mybir.dt.float32
    bf16 = mybir.dt.bfloat16
    Exp = mybir.ActivationFunctionType.Exp
    GE = mybir.AluOpType.is_ge

    nc = bacc.Bacc("TRN2", target_bir_lowering=False, debug=False, num_devices=NCORES)

    x_d = nc.declare_dram_parameter("x", [BPC, T, C], f32, isOutput=False)
    wa_d = nc.declare_dram_parameter("W_attn", [C, 3 * C], f32, isOutput=False)
    wp_d = nc.declare_dram_parameter("W_proj", [C, C], f32, isOutput=False)
    bp_d = nc.declare_dram_parameter("b_proj", [C], f32, isOutput=False)
    out_d = nc.declare_dram_parameter("out", [BPC, T, C], f32, isOutput=True)

    # virtual clocks (ns) for PE / ACT emission balancing
    est = {"pe": 0.0, "act": 0.0}

    def MM(n):                      # matmul cost, cols n
        return n / 2.4 + 10.0

    def EXPC(n):                    # ACT exp cost, free-elems n
        return n * 1.0 + 250.0

    def bump_pe(cost):
        est["pe"] = min(est["pe"] + cost, est["act"] + VM_CAP)

    with TileContext(nc) as tc:
        with (
            tc.tile_pool(name="const", bufs=1) as const,
            tc.tile_pool(name="xin", bufs=2) as xin,
            tc.tile_pool(name="xT", bufs=2) as xTp,
            tc.tile_pool(name="qk", bufs=2) as qkp,
            tc.tile_pool(name="vb", bufs=2) as vbp,
            tc.tile_pool(name="pT", bufs=2) as pTp,
            tc.tile_pool(name="yn", bufs=2) as ynp,
            tc.tile_pool(name="yT", bufs=2) as yTp,
            tc.tile_pool(name="rc", bufs=4) as rcp,
            tc.tile_pool(name="osb", bufs=2) as osbp,
            tc.tile_pool(name="psA", bufs=2, space="PSUM") as psA,   # 1 bank each
            tc.tile_pool(name="psS", bufs=2, space="PSUM") as psS,   # 2 banks each
            tc.tile_pool(name="psY", bufs=1, space="PSUM") as psY,   # 2 banks
        ):
            # ---- weights via cast-DMA (bf16 in flight) ----
            wa_sb = []
            wp_sb = []
            for c in range(3):
                w = const.tile([128, 3 * C], bf16, tag=f"wa{c}")
                nc.gpsimd.dma_start(out=w[:], in_=wa_d[c * 128:(c + 1) * 128, :])
                wa_sb.append(w)
                p = const.tile([128, C], bf16, tag=f"wp{c}")
                nc.gpsimd.dma_start(out=p[:], in_=wp_d[c * 128:(c + 1) * 128, :])
                wp_sb.append(p)
            b_bc = const.tile([128, C], f32, tag="bbc")
            nc.sync.dma_start(
                out=b_bc[:], in_=bp_d[:].unsqueeze(0).broadcast_to([128, C])
            )

            def load(b):
                """Cast-DMA x[b] f32 DRAM -> bf16 SBUF, then crossbar x^T."""
                xb = xin.tile([128, TT * C], bf16, tag="xin", name=f"xb{b}")
                nc.gpsimd.dma_start(
                    out=xb[:].rearrange("p (t c) -> p t c", t=TT),
                    in_=x_d[b].rearrange("(t p) c -> p t c", p=128),
                )
                xT = xTp.tile([128, 3 * T], bf16, tag="xT", name=f"xT{b}")
                for t in range(TT):
                    nc.sync.dma_start_transpose(
                        xT[:].rearrange("p (c t) -> p c t", c=3)
                        [:, :, t * 128:(t + 1) * 128],
                        xb[:, t * C:(t + 1) * C],
                    )
                return xT

            def emit_v(b, xT, vb, t):
                psv = psA.tile([128, 512], f32, tag="psA", name=f"psv{b}")
                for c in range(3):
                    nc.tensor.matmul(
                        psv[:, 0:C],
                        lhsT=xT[:, c * T + t * 128: c * T + t * 128 + 128],
                        rhs=wa_sb[c][:, 2 * C: 3 * C],
                        start=(c == 0),
                        stop=(c == 2),
                    )
                nc.vector.tensor_copy(
                    vb[:, t * VSTRIDE: t * VSTRIDE + VSTRIDE]
                    .rearrange("p (h e) -> p h e", e=D + 1)[:, :, 0:D],
                    psv[:, 0:C].rearrange("p (h d) -> p h d", d=D),
                )

            def emit_qk(b, xT, qk, i, m, half):
                psq = psA.tile([128, 512], f32, tag="psA", name=f"psq{b}")
                for c in range(3):
                    nc.tensor.matmul(
                        psq[:],
                        lhsT=wa_sb[c][:, m * 128:(m + 1) * 128],
                        rhs=xT[:, c * T + half * 512: c * T + half * 512 + 512],
                        start=(c == 0),
                        stop=(c == 2),
                    )
                nc.vector.tensor_copy(
                    qk[:, i * T + half * 512: i * T + half * 512 + 512],
                    psq[:],
                )

            def prep_fillers(b, xT):
                """(vb, qks, early, late) filler lists for batch b.

                Early (deadline 2b):   memset + all k^T + q^T half0 + v 0-3.
                Late  (deadline 2b+1): q^T half1 + v 4-7.
                Entry: (pe_cost_ns, closure, deadline).
                """
                vb = vbp.tile([128, TT * VSTRIDE], bf16, tag="vb", name=f"vb{b}")
                qks = [qkp.tile([128, 2 * T], bf16, tag=f"qk{pp}",
                                name=f"qk{b}_{pp}") for pp in range(NPAIR)]

                def head():
                    nc.gpsimd.memset(
                        vb[:].rearrange("p (t h e) -> p t h e", t=TT, e=D + 1)
                        [:, :, :, D:],
                        1.0,
                    )
                d0, d1 = 2 * b, 2 * b + 1
                early = [(0.0, head, d0)]
                qcost = 3 * MM(512)
                vcost = 3 * MM(384)
                eq = []
                for pp in range(NPAIR):
                    eq.append((qcost, lambda pp=pp:
                               emit_qk(b, xT, qks[pp], 0, pp, 0), d0))    # q half0
                    for half in range(2):
                        eq.append((qcost, lambda pp=pp, half=half:
                                   emit_qk(b, xT, qks[pp], 1, 3 + pp, half), d0))
                ev = [(vcost, lambda t=t: emit_v(b, xT, vb, t), d0)
                      for t in range(4)]
                while ev or eq:
                    if ev:
                        early.append(ev.pop(0))
                    if eq:
                        early.append(eq.pop(0))
                    if eq:
                        early.append(eq.pop(0))
                late = []
                for pp in range(NPAIR):
                    late.append((qcost, lambda pp=pp:
                                 emit_qk(b, xT, qks[pp], 0, pp, 1), d1))  # q half1
                    late.append((vcost, lambda t=4 + pp:
                                 emit_v(b, xT, vb, t), d1))
                late.append((vcost, lambda: emit_v(b, xT, vb, 7), d1))
                return vb, qks, early, late

            def emit_proj(b, yT, t):
                pso = psA.tile([128, 512], f32, tag="psA", name=f"pso{b}")
                for c in range(3):
                    nc.tensor.matmul(
                        pso[:, 0:C],
                        lhsT=yT[:, c * T + t * 128: c * T + t * 128 + 128],
                        rhs=wp_sb[c][:],
                        start=(c == 0),
                        stop=(c == 2),
                    )
                osb = osbp.tile([128, C], f32, tag="osb", name=f"osb{b}")
                nc.vector.tensor_add(osb[:], pso[:, 0:C], b_bc[:])
                nc.sync.dma_start(
                    out=out_d[b, t * 128:(t + 1) * 128, :], in_=osb[:]
                )

            # ---- filler machinery driven by the virtual clocks ----
            fillers = []

            def fill_until():
                while fillers and est["pe"] < est["act"] + VM_MARGIN:
                    cost, f, _ = fillers.pop(0)
                    f()
                    bump_pe(cost)

            def force_deadline(d):
                rest = []
                for cost, f, dl in fillers:
                    if dl <= d:
                        f()
                        bump_pe(cost)
                    else:
                        rest.append((cost, f, dl))
                fillers[:] = rest

            def drain_fillers():
                while fillers:
                    cost, f, _ = fillers.pop(0)
                    f()
                    bump_pe(cost)

            def attn(b, vb, qks):
                """Attention for batch b -> proj outputs (as deferred fillers)."""
                yT = yTp.tile([128, 3 * T], bf16, tag="yT", name=f"yT{b}")
                for qc in range(QC):
                    force_deadline(2 * b + qc)
                    yn = ynp.tile([128, 4 * C], bf16, tag=f"yn{qc}",
                                  name=f"yn{b}_{qc}")
                    for p in range(NPAIR):
                        qk = qks[p]
                        nkt = 4 * (qc + 1)
                        ptw = nkt * 512
                        pT = pTp.tile([128, 2 * ptw], bf16, tag=f"pT{qc}",
                                      name=f"pT{b}_{p}_{qc}")
                        ys = psY.tile([128, 1024], f32, tag="psY",
                                      name=f"ys{b}{p}{qc}")

                        def ycol(j, hh):
                            return (512 if j == 3 else j * 130) + hh * 65

                        def emit_scores_exp(kt):
                            diag = kt >= qc * 4
                            o = (kt - qc * 4) * 128 if diag else 0
                            pss = psS.tile([128, 1024], f32, tag="psS",
                                           name=f"pss{b}{p}")
                            for hh in range(2):
                                nc.tensor.matmul(
                                    pss[:, hh * 512 + o:(hh + 1) * 512],
                                    lhsT=qk[hh * 64:(hh + 1) * 64,
                                            T + kt * 128: T + kt * 128 + 128],
                                    rhs=qk[hh * 64:(hh + 1) * 64,
                                           qc * 512 + o: qc * 512 + 512],
                                    start=True,
                                    stop=True,
                                )
                            nc.scalar.activation(
                                pT[:].rearrange("p (h w) -> p h w", h=2)
                                [:, :, kt * 512 + o: (kt + 1) * 512],
                                pss[:].rearrange("p (h w) -> p h w", h=2)
                                [:, :, o:512],
                                Exp,
                                scale=0.125,
                            )
                            bump_pe(2 * MM(512 - o))
                            est["act"] += EXPC(2 * (512 - o))
                            if diag:
                                blk = pT[:].rearrange("p (h w) -> p h w", h=2)[
                                    :, :, kt * 512 + o: kt * 512 + o + 128]
                                nc.gpsimd.affine_select(
                                    out=blk,
                                    in_=blk,
                                    compare_op=GE,
                                    fill=0.0,
                                    base=0,
                                    pattern=[[0, 2], [1, 128]],
                                    channel_multiplier=-1,
                                )

                        def emit_y(j):
                            # one accumulation chain per (j, hh) psum region;
                            # chains strictly sequential within a psum bank
                            qt = qc * 4 + j
                            for hh in range(2):
                                h = 2 * p + hh
                                for k2 in range(qt + 1):
                                    nc.tensor.matmul(
                                        ys[:, ycol(j, hh): ycol(j, hh) + 65],
                                        lhsT=pT[:, hh * ptw + k2 * 512 + j * 128:
                                                hh * ptw + k2 * 512 + j * 128 + 128],
                                        rhs=vb[:, k2 * VSTRIDE + h * (D + 1):
                                               k2 * VSTRIDE + (h + 1) * (D + 1)],
                                        start=(k2 == 0),
                                        stop=(k2 == qt),
                                    )
                            bump_pe(2 * (qt + 1) * 37.0)

                        emit_scores_exp(0)
                        fill_until()
                        for kt in range(1, nkt):
                            emit_scores_exp(kt)
                            fill_until()
                            if kt - 1 >= qc * 4:
                                emit_y(kt - 1 - qc * 4)
                        emit_y(3)

                        rc = rcp.tile([128, 8], f32, tag="rc",
                                      name=f"rc{b}{p}{qc}")
                        nc.vector.reciprocal(rc[:, 0:6], ys[:, 64:454:65])
                        nc.vector.reciprocal(rc[:, 6:8], ys[:, 576:706:65])
                        nc.vector.tensor_mul(
                            yn[:, 0: 3 * C]
                            .rearrange("p (j w) -> p j w", j=3)
                            [:, :, 2 * p * 64: 2 * p * 64 + 128]
                            .rearrange("p j (g e) -> p j g e", e=D),
                            ys[:, 0:390]
                            .rearrange("p (j g e) -> p j g e", g=2, e=D + 1)
                            [:, :, :, 0:D],
                            rc[:, 0:6]
                            .rearrange("p (j g) -> p j g", g=2)
                            .unsqueeze(3).broadcast_to([128, 3, 2, D]),
                        )
                        nc.vector.tensor_mul(
                            yn[:, 3 * C + 2 * p * 64: 3 * C + 2 * p * 64 + 128]
                            .rearrange("p (g e) -> p g e", e=D),
                            ys[:, 512:642]
                            .rearrange("p (g e) -> p g e", e=D + 1)[:, :, 0:D],
                            rc[:, 6:8].unsqueeze(2).broadcast_to([128, 2, D]),
                        )
                        fill_until()

                    # y^T for this qc via crossbar; projections become
                    # deferred fillers so the crossbar latency hides behind
                    # the next score phase
                    for j in range(4):
                        qt = qc * 4 + j
                        nc.sync.dma_start_transpose(
                            yT[:].rearrange("p (c t) -> p c t", c=3)
                            [:, :, qt * 128: qt * 128 + 128],
                            yn[:, j * C:(j + 1) * C],
                        )
                        fillers.append(
                            (3 * MM(384),
                             lambda t=qt: emit_proj(b, yT, t),
                             2 * (b + 1) + qc)
                        )

            # ---- schedule ----
            xT0 = load(0)
            vb0, qks0, early0, late0 = prep_fillers(0, xT0)
            for cost, f, _ in early0:
                f()
                est["pe"] += cost
            xT_next = load(1)

            vb_cur, qks_cur, late_cur = vb0, qks0, late0
            for b in range(BPC):
                fillers[:0] = late_cur
                if b + 1 < BPC:
                    vb_nxt, qks_nxt, early_n, late_n = prep_fillers(b + 1, xT_next)
                    fillers.extend(early_n)
                attn(b, vb_cur, qks_cur)
                # after attn(b): all of xT(b)'s readers (late fillers) are
                # emitted, so the xT buffer may rotate to batch b+2
                if b + 2 < BPC:
                    xT_next = load(b + 2)
                if b + 1 < BPC:
                    vb_cur, qks_cur, late_cur = vb_nxt, qks_nxt, late_n
            drain_fillers()

    nc.finalize()
    return nc


def _run(inputs, trace=False, **kw):
    from concourse.bass_utils import run_bass_kernel_spmd

    if "nc" not in _nc_cache:
        _nc_cache["nc"] = _build_nc()
    nc = _nc_cache["nc"]

    x = np.ascontiguousarray(np.asarray(inputs["x"], dtype=np.float32))
    wa = np.ascontiguousarray(np.asarray(inputs["W_attn"], dtype=np.float32))
    wp = np.ascontiguousarray(np.asarray(inputs["W_proj"], dtype=np.float32))
    bp = np.ascontiguousarray(np.asarray(inputs["b_proj"], dtype=np.float32))

    in_maps = [
        {"x": x[i * BPC:(i + 1) * BPC], "W_attn": wa, "W_proj": wp, "b_proj": bp}
        for i in range(NCORES)
    ]
    res = run_bass_kernel_spmd(nc, in_maps, list(range(NCORES)), trace=trace, **kw)
    out = np.concatenate([res.results[i]["out"] for i in range(NCORES)], axis=0)
    return out, res


def kernel(**inputs) -> np.ndarray:
    out, _ = _run(inputs, trace=False)
    return out


# revision 5
# speedup vs baseline: 1.0792x; 1.0469x over previous
"""Multi-head causal self-attention block for Trainium2, data-parallel over 8 cores.

Reference computation (per batch b of x [B=32, T=1024, C=384]):
    qkv = x @ W_attn;  q,k,v heads (H=6, D=64)
    y   = softmax(causal(q k^T / sqrt(D))) @ v
    out = y @ W_proj + b_proj
Sharding: batch dim 32 -> 4 per core, weights replicated, no collectives.

v5 design (evolves v3/v4; bf16 matmul operands, f32 psum accumulation):
  - x loaded f32 via hwdge DMA, cast to bf16 per 128-token tile on gpsimd
    (sbuf->sbuf), then x^T built by DMA crossbar transposes
    (dma_start_transpose): the PE does no transposes at all and the DVE
    does no xT evictions. Each cast+xbar pair is a paced filler so the
    in-order gpsimd queue never blocks the attention affine_selects.
  - q^T,k^T per head-pair [128, T] (W_attn slices stationary); v natural
    [tok, 384] per k-tile with a bf16 ones column per head (softmax
    denominator rides the AV matmul for free).
  - scores transposed s^T[k,q] per 512-wide q chunk, exp on ACT (scale=1/8)
    into bf16 pT; causal diagonal fixed by gpsimd affine_select.
  - AV in NATURAL layout: out[q, d+1] accumulated per 128-q chunk; softmax
    normalization = strided DVE reciprocal + broadcast-AP multiplies.
  - y natural -> y^T via DMA crossbar transposes (no PE/DVE involvement);
    proj runs as deferred filler work (with a readiness delay) so the
    crossbar latency hides behind the next q-chunk's score phase; bias
    added on the DVE eviction.
  - VIRTUAL-CLOCK INTERLEAVE: emission tracks estimated cumulative PE and
    ACT busy-ns; filler chunks (next batch's cast/xbar/v/qk, deferred
    proj) are inserted whenever the PE stream would fall behind the ACT
    stream, keeping the in-order PE queue dense so the tensor engine
    stays ramped at its top p-state. Fillers carry deadline tags (global
    qc index; force-emitted when their consumer phase begins) and a
    readiness gate (min ACT-clock before emission). Per-batch prep is
    split into an "early" part (needed before the batch's qc0 attention)
    and a "late" part (only needed by qc1), widening the filler window.
"""

import sys

if "/opt/trn_rl_repo" not in sys.path:
    sys.path.insert(0, "/opt/trn_rl_repo")

import numpy as np

B, T, C = 32, 1024, 384
H, D = 6, 64
NCORES = 8
BPC = B // NCORES          # batches per core
NPAIR = H // 2             # head pairs
TT = T // 128              # token tiles per batch (8)
QC = T // 512              # q chunks per batch (2)
VSTRIDE = H * (D + 1)      # 390: per-token-tile v_aug row width

VM_MARGIN = 500.0          # ns of PE lead to maintain over ACT
VM_CAP = 1500.0            # max bankable PE lead (queue depth model)
PROJ_DELAY = 2500.0        # ns of ACT-clock before a deferred proj is ready

_nc_cache = {}


def _build_nc():
    import concourse.mybir as mybir
    from concourse import bacc
    from concourse.tile import TileContext

    f32 = mybir.dt.float32
    bf16 = mybir.dt.bfloat16
    Exp = mybir.ActivationFunctionType.Exp
    GE = mybir.AluOpType.is_ge

    nc = bacc.Bacc("TRN2", target_bir_lowering=False, debug=False, num_devices=NCORES)

    x_d = nc.declare_dram_parameter("x", [BPC, T, C], f32, isOutput=False)
    wa_d = nc.declare_dram_parameter("W_attn", [C, 3 * C], f32, isOutput=False)
    wp_d = nc.declare_dram_parameter("W_proj", [C, C], f32, isOutput=False)
    bp_d = nc.declare_dram_parameter("b_proj", [C], f32, isOutput=False)
    out_d = nc.declare_dram_parameter("out", [BPC, T, C], f32, isOutput=True)

    # virtual clocks (ns) for PE / ACT emission balancing
    est = {"pe": 0.0, "act": 0.0}

    def MM(n):                      # matmul cost, cols n
        return n / 2.4 + 10.0

    def EXPC(n):                    # ACT exp cost, free-elems n
        return n * 1.0 + 250.0

    def bump_pe(cost):
        est["pe"] = min(est["pe"] + cost, est["act"] + VM_CAP)

    with TileContext(nc) as tc:
        with (
            tc.tile_pool(name="const", bufs=1) as const,
            tc.tile_pool(name="xf", bufs=2) as xfp,
            tc.tile_pool(name="xb", bufs=2) as xbp,
            tc.tile_pool(name="xT", bufs=2) as xTp,
            tc.tile_pool(name="qk", bufs=2) as qkp,
            tc.tile_pool(name="vb", bufs=2) as vbp,
            tc.tile_pool(name="pT", bufs=2) as pTp,
            tc.tile_pool(name="yn", bufs=2) as ynp,
            tc.tile_pool(name="yT", bufs=2) as yTp,
            tc.tile_pool(name="rc", bufs=4) as rcp,
            tc.tile_pool(name="osb", bufs=2) as osbp,
            tc.tile_pool(name="psA", bufs=2, space="PSUM") as psA,   # 1 bank each
            tc.tile_pool(name="psS", bufs=2, space="PSUM") as psS,   # 2 banks each
            tc.tile_pool(name="psY", bufs=1, space="PSUM") as psY,   # 2 banks
        ):
            # ---- weights: hwdge f32 DMA + DVE cast (fast prologue) ----
            wa_sb = []
            wp_sb = []
            wstg = []
            for c in range(3):
                wf = const.tile([128, 4 * C], f32, tag=f"wf{c}")
                nc.sync.dma_start(out=wf[:, 0: 3 * C],
                                  in_=wa_d[c * 128:(c + 1) * 128, :])
                nc.sync.dma_start(out=wf[:, 3 * C: 4 * C],
                                  in_=wp_d[c * 128:(c + 1) * 128, :])
                wstg.append(wf)
            for c in range(3):
                w = const.tile([128, 3 * C], bf16, tag=f"wa{c}")
                nc.vector.tensor_copy(w[:], wstg[c][:, 0: 3 * C])
                wa_sb.append(w)
                p = const.tile([128, C], bf16, tag=f"wp{c}")
                nc.vector.tensor_copy(p[:], wstg[c][:, 3 * C: 4 * C])
                wp_sb.append(p)
            b_bc = const.tile([128, C], f32, tag="bbc")
            nc.sync.dma_start(
                out=b_bc[:], in_=bp_d[:].unsqueeze(0).broadcast_to([128, C])
            )

            def load(b):
                """x[b] f32 DRAM -> SBUF staging via hwdge DMA (8 t-tiles)."""
                xf = xfp.tile([128, TT * C], f32, tag="xf", name=f"xf{b}")
                for t in range(TT):
                    nc.sync.dma_start(
                        out=xf[:, t * C:(t + 1) * C],
                        in_=x_d[b, t * 128:(t + 1) * 128, :],
                    )
                return xf

            def emit_v(b, xT, vb, t):
                psv = psA.tile([128, 512], f32, tag="psA", name=f"psv{b}")
                for c in range(3):
                    nc.tensor.matmul(
                        psv[:, 0:C],
                        lhsT=xT[:, c * T + t * 128: c * T + t * 128 + 128],
                        rhs=wa_sb[c][:, 2 * C: 3 * C],
                        start=(c == 0),
                        stop=(c == 2),
                    )
                nc.vector.tensor_copy(
                    vb[:, t * VSTRIDE: t * VSTRIDE + VSTRIDE]
                    .rearrange("p (h e) -> p h e", e=D + 1)[:, :, 0:D],
                    psv[:, 0:C].rearrange("p (h d) -> p h d", d=D),
                )

            def emit_qk(b, xT, qk, i, m, half):
                psq = psA.tile([128, 512], f32, tag="psA", name=f"psq{b}")
                for c in range(3):
                    nc.tensor.matmul(
                        psq[:],
                        lhsT=wa_sb[c][:, m * 128:(m + 1) * 128],
                        rhs=xT[:, c * T + half * 512: c * T + half * 512 + 512],
                        start=(c == 0),
                        stop=(c == 2),
                    )
                nc.vector.tensor_copy(
                    qk[:, i * T + half * 512: i * T + half * 512 + 512],
                    psq[:],
                )

            def prep_fillers(b, xf):
                """(vb, qks, early, late) filler lists for batch b.

                Early (deadline 2b):   memset + per-tile cast+xbar + all k^T
                                       + q^T half0 + v 0-3.
                Late  (deadline 2b+1): q^T half1 + v 4-7.
                Entry: (pe_cost_ns, closure, deadline, ready_act).
                """
                vb = vbp.tile([128, TT * VSTRIDE], bf16, tag="vb", name=f"vb{b}")
                qks = [qkp.tile([128, 2 * T], bf16, tag=f"qk{pp}",
                                name=f"qk{b}_{pp}") for pp in range(NPAIR)]
                xb = xbp.tile([128, TT * C], bf16, tag="xb", name=f"xb{b}")
                xT = xTp.tile([128, 3 * T], bf16, tag="xT", name=f"xT{b}")

                def head():
                    nc.gpsimd.memset(
                        vb[:].rearrange("p (t h e) -> p t h e", t=TT, e=D + 1)
                        [:, :, :, D:],
                        1.0,
                    )

                def cast_xbar(t):
                    nc.gpsimd.tensor_copy(
                        xb[:, t * C:(t + 1) * C], xf[:, t * C:(t + 1) * C]
                    )
                    nc.sync.dma_start_transpose(
                        xT[:].rearrange("p (c t) -> p c t", c=3)
                        [:, :, t * 128:(t + 1) * 128],
                        xb[:, t * C:(t + 1) * C],
                    )
                d0, d1 = 2 * b, 2 * b + 1
                early = [(0.0, head, d0, 0.0)]
                for t in range(TT):
                    early.append((150.0, lambda t=t: cast_xbar(t), d0, 0.0))
                qcost = 3 * MM(512)
                vcost = 3 * MM(384)
                eq = []
                for pp in range(NPAIR):
                    eq.append((qcost, lambda pp=pp:
                               emit_qk(b, xT, qks[pp], 0, pp, 0), d0, 0.0))
                    for half in range(2):
                        eq.append((qcost, lambda pp=pp, half=half:
                                   emit_qk(b, xT, qks[pp], 1, 3 + pp, half),
                                   d0, 0.0))
                ev = [(vcost, lambda t=t: emit_v(b, xT, vb, t), d0, 0.0)
                      for t in range(4)]
                while ev or eq:
                    if ev:
                        early.append(ev.pop(0))
                    if eq:
                        early.append(eq.pop(0))
                    if eq:
                        early.append(eq.pop(0))
                late = []
                for pp in range(NPAIR):
                    late.append((qcost, lambda pp=pp:
                                 emit_qk(b, xT, qks[pp], 0, pp, 1), d1, 0.0))
                    late.append((vcost, lambda t=4 + pp:
                                 emit_v(b, xT, vb, t), d1, 0.0))
                late.append((vcost, lambda: emit_v(b, xT, vb, 7), d1, 0.0))
                return vb, qks, early, late

            def emit_proj(b, yT, t):
                pso = psA.tile([128, 512], f32, tag="psA", name=f"pso{b}")
                for c in range(3):
                    nc.tensor.matmul(
                        pso[:, 0:C],
                        lhsT=yT[:, c * T + t * 128: c * T + t * 128 + 128],
                        rhs=wp_sb[c][:],
                        start=(c == 0),
                        stop=(c == 2),
                    )
                osb = osbp.tile([128, C], f32, tag="osb", name=f"osb{b}")
                nc.vector.tensor_add(osb[:], pso[:, 0:C], b_bc[:])
                nc.sync.dma_start(
                    out=out_d[b, t * 128:(t + 1) * 128, :], in_=osb[:]
                )

            # ---- filler machinery driven by the virtual clocks ----
            fillers = []

            def fill_until():
                while fillers and est["pe"] < est["act"] + VM_MARGIN:
                    hit = None
                    for idx, (cost, f, dl, ready) in enumerate(fillers):
                        if ready <= est["act"]:
                            hit = idx
                            break
                    if hit is None:
                        return
                    cost, f, dl, ready = fillers.pop(hit)
                    f()
                    bump_pe(cost)

            def force_deadline(d):
                rest = []
                for cost, f, dl, ready in fillers:
                    if dl <= d:
                        f()
                        bump_pe(cost)
                    else:
                        rest.append((cost, f, dl, ready))
                fillers[:] = rest

            def drain_fillers():
                while fillers:
                    cost, f, _, _ = fillers.pop(0)
                    f()
                    bump_pe(cost)

            def attn(b, vb, qks):
                """Attention for batch b; projections left as deferred fillers."""
                yT = yTp.tile([128, 3 * T], bf16, tag="yT", name=f"yT{b}")
                for qc in range(QC):
                    force_deadline(2 * b + qc)
                    yn = ynp.tile([128, 4 * C], bf16, tag=f"yn{qc}",
                                  name=f"yn{b}_{qc}")
                    for p in range(NPAIR):
                        qk = qks[p]
                        nkt = 4 * (qc + 1)
                        ptw = nkt * 512
                        pT = pTp.tile([128, 2 * ptw], bf16, tag=f"pT{qc}",
                                      name=f"pT{b}_{p}_{qc}")
                        ys = psY.tile([128, 1024], f32, tag="psY",
                                      name=f"ys{b}{p}{qc}")

                        def ycol(j, hh):
                            return (512 if j == 3 else j * 130) + hh * 65

                        def emit_scores_exp(kt):
                            diag = kt >= qc * 4
                            o = (kt - qc * 4) * 128 if diag else 0
                            pss = psS.tile([128, 1024], f32, tag="psS",
                                           name=f"pss{b}{p}")
                            for hh in range(2):
                                nc.tensor.matmul(
                                    pss[:, hh * 512 + o:(hh + 1) * 512],
                                    lhsT=qk[hh * 64:(hh + 1) * 64,
                                            T + kt * 128: T + kt * 128 + 128],
                                    rhs=qk[hh * 64:(hh + 1) * 64,
                                           qc * 512 + o: qc * 512 + 512],
                                    start=True,
                                    stop=True,
                                )
                            nc.scalar.activation(
                                pT[:].rearrange("p (h w) -> p h w", h=2)
                                [:, :, kt * 512 + o: (kt + 1) * 512],
                                pss[:].rearrange("p (h w) -> p h w", h=2)
                                [:, :, o:512],
                                Exp,
                                scale=0.125,
                            )
                            bump_pe(2 * MM(512 - o))
                            est["act"] += EXPC(2 * (512 - o))
                            if diag:
                                blk = pT[:].rearrange("p (h w) -> p h w", h=2)[
                                    :, :, kt * 512 + o: kt * 512 + o + 128]
                                nc.gpsimd.affine_select(
                                    out=blk,
                                    in_=blk,
                                    compare_op=GE,
                                    fill=0.0,
                                    base=0,
                                    pattern=[[0, 2], [1, 128]],
                                    channel_multiplier=-1,
                                )

                        def emit_y(j):
                            # one accumulation chain per (j, hh) psum region;
                            # chains strictly sequential within a psum bank
                            qt = qc * 4 + j
                            for hh in range(2):
                                h = 2 * p + hh
                                for k2 in range(qt + 1):
                                    nc.tensor.matmul(
                                        ys[:, ycol(j, hh): ycol(j, hh) + 65],
                                        lhsT=pT[:, hh * ptw + k2 * 512 + j * 128:
                                                hh * ptw + k2 * 512 + j * 128 + 128],
                                        rhs=vb[:, k2 * VSTRIDE + h * (D + 1):
                                               k2 * VSTRIDE + (h + 1) * (D + 1)],
                                        start=(k2 == 0),
                                        stop=(k2 == qt),
                                    )
                            bump_pe(2 * (qt + 1) * 45.0)

                        emit_scores_exp(0)
                        fill_until()
                        for kt in range(1, nkt):
                            emit_scores_exp(kt)
                            fill_until()
                            if kt - 1 >= qc * 4:
                                emit_y(kt - 1 - qc * 4)
                        emit_y(3)

                        rc = rcp.tile([128, 8], f32, tag="rc",
                                      name=f"rc{b}{p}{qc}")
                        nc.vector.reciprocal(rc[:, 0:6], ys[:, 64:454:65])
                        nc.vector.reciprocal(rc[:, 6:8], ys[:, 576:706:65])
                        nc.vector.tensor_mul(
                            yn[:, 0: 3 * C]
                            .rearrange("p (j w) -> p j w", j=3)
                            [:, :, 2 * p * 64: 2 * p * 64 + 128]
                            .rearrange("p j (g e) -> p j g e", e=D),
                            ys[:, 0:390]
                            .rearrange("p (j g e) -> p j g e", g=2, e=D + 1)
                            [:, :, :, 0:D],
                            rc[:, 0:6]
                            .rearrange("p (j g) -> p j g", g=2)
                            .unsqueeze(3).broadcast_to([128, 3, 2, D]),
                        )
                        nc.vector.tensor_mul(
                            yn[:, 3 * C + 2 * p * 64: 3 * C + 2 * p * 64 + 128]
                            .rearrange("p (g e) -> p g e", e=D),
                            ys[:, 512:642]
                            .rearrange("p (g e) -> p g e", e=D + 1)[:, :, 0:D],
                            rc[:, 6:8].unsqueeze(2).broadcast_to([128, 2, D]),
                        )
                        fill_until()

                    # y^T for this qc via crossbar; projections become
                    # deferred fillers (with readiness delay) so the
                    # crossbar latency hides behind the next score phase
                    for j in range(4):
                        qt = qc * 4 + j
                        nc.sync.dma_start_transpose(
                            yT[:].rearrange("p (c t) -> p c t", c=3)
                            [:, :, qt * 128: qt * 128 + 128],
                            yn[:, j * C:(j + 1) * C],
                        )
                        fillers.append(
                            (3 * MM(384),
                             lambda t=qt: emit_proj(b, yT, t),
                             2 * (b + 1) + qc,
                             est["act"] + PROJ_DELAY)
                        )

            # ---- schedule ----
            xf0 = load(0)
            vb0, qks0, early0, late0 = prep_fillers(0, xf0)
            for cost, f, _, _ in early0:
                f()
                est["pe"] += cost
            xf_next = load(1)

            vb_cur, qks_cur, late_cur = vb0, qks0, late0
            for b in range(BPC):
                if b + 1 < BPC:
                    vb_nxt, qks_nxt, early_n, late_n = prep_fillers(b + 1, xf_next)
                else:
                    early_n = []
                # interleave late(b) between the cast/xbar entries of
                # early(b+1) so the first v/qk of b+1 never waits on a
                # just-issued crossbar
                merged = []
                la, ea = list(late_cur), list(early_n)
                while la or ea:
                    if ea:
                        merged.append(ea.pop(0))
                    if la:
                        merged.append(la.pop(0))
                fillers.extend(merged)
                if b + 2 < BPC:
                    xf_next = load(b + 2)
                attn(b, vb_cur, qks_cur)
                if b + 1 < BPC:
                    vb_cur, qks_cur, late_cur = vb_nxt, qks_nxt, late_n
            drain_fillers()

    nc.finalize()
    return nc


def _run(inputs, trace=False, **kw):
    from concourse.bass_utils import run_bass_kernel_spmd

    if "nc" not in _nc_cache:
        _nc_cache["nc"] = _build_nc()
    nc = _nc_cache["nc"]

    x = np.ascontiguousarray(np.asarray(inputs["x"], dtype=np.float32))
    wa = np.ascontiguousarray(np.asarray(inputs["W_attn"], dtype=np.float32))
    wp = np.ascontiguousarray(np.asarray(inputs["W_proj"], dtype=np.float32))
    bp = np.ascontiguousarray(np.asarray(inputs["b_proj"], dtype=np.float32))

    in_maps = [
        {"x": x[i * BPC:(i + 1) * BPC], "W_attn": wa, "W_proj": wp, "b_proj": bp}
        for i in range(NCORES)
    ]
    res = run_bass_kernel_spmd(nc, in_maps, list(range(NCORES)), trace=trace, **kw)
    out = np.concatenate([res.results[i]["out"] for i in range(NCORES)], axis=0)
    return out, res


def kernel(**inputs) -> np.ndarray:
    out, _ = _run(inputs, trace=False)
    return out
